# revision 3
# baseline (speedup 1.0000x reference)
"""Sharded cosine-similarity kNN retrieval kernel for Trainium2 (Bass/Tile).

Problem: one query [D] against keys [N, D]; return actions[top_k indices of
cosine similarity].  N=100000, D=2048, A=7, top_k<=8.

Strategy (v2, TensorEngine Gram):
  - Shard keys row-wise across 8 NeuronCores.  Each core's shard is stored
    HOST-TRANSPOSED as an fp16 buffer kaug [D=2048, 99*128] where each
    128-column block is [q | 127 key rows] (q embedded at column 0 of every
    block by the host).
  - Per 128-column block B (viewed per 128-d chunk c as [128, 128] SBUF
    tiles with d on partitions): a single accumulation group of 16 matmuls
      psum += B_c.T @ B_c          (lhsT = rhs = B_c)
    yields, in one PE stream, BOTH
      psum[m, 0]   = <col_m, q>    -> dots   (column 0)
      psum[m, m]   = ||col_m||^2   -> norms2 (diagonal)
    TensorE does all the O(N*D) work; DVE extracts the diagonal via an
    identity-mask multiply-accumulate, ACT copies the dots column.  The
    kernel is then HBM-bandwidth bound (~51 MB/core fp16).
  - Host: sims = dots / max(|k| * |q|, eps), global top-k over 100k scalars,
    gather actions rows (the standard "reduce M*k candidates" step).
"""

import sys

for _p in ("/opt/trn_rl_repo", "/opt/trn_rl_repo/concourse"):
    if _p not in sys.path:
        sys.path.insert(0, _p)

import numpy as np

import concourse.bacc as bacc
from concourse import mybir
from concourse.bass_utils import run_bass_kernel_spmd
from concourse.masks import make_identity
from concourse.tile import TileContext

N, D, A = 100000, 2048, 7
EPS = 1e-8
N_CORES = 8
P = 128
C = D // P                       # 16 d-chunks
ROWS_PER_CORE = 12544            # 8*12544 = 100352 >= N
RPB = 127                        # key rows per 128-col block (col 0 is q)
BLOCKS = 99                      # ceil(12544/127); 99*127 = 12573 row slots
COLS = BLOCKS * P                # 12672 columns in the augmented buffer
PANEL_BLOCKS = 3                 # blocks per DMA panel
PANELS = BLOCKS // PANEL_BLOCKS  # 33
USE_FP16 = True

_CACHE = {}


def _build_bass(repeats: int = 1, fp16: bool = USE_FP16,
                panel_blocks: int = PANEL_BLOCKS):
    """Build the per-core Bass program.

    repeats>1 wraps the streaming loop in a hardware For loop that re-reads
    the same DRAM shard; used only for wall-clock HW timing (slope over
    repeats cancels host/axon dispatch overhead)."""
    assert BLOCKS % panel_blocks == 0
    panels = BLOCKS // panel_blocks
    pw = panel_blocks * P        # panel width in columns

    nc = bacc.Bacc(
        "TRN2",
        target_bir_lowering=False,
        debug=False,
        enable_asserts=False,
        num_devices=N_CORES,
    )
    f32 = mybir.dt.float32
    kdt = mybir.dt.float16 if fp16 else f32
    kaug_d = nc.dram_tensor(
        "kaug", [D, COLS], kdt, kind="ExternalInput"
    ).ap()
    dots_d = nc.dram_tensor(
        "dots", [P, BLOCKS], f32, kind="ExternalOutput"
    ).ap()
    norms2_d = nc.dram_tensor(
        "norms2", [P, BLOCKS], f32, kind="ExternalOutput"
    ).ap()

    # kaug viewed as [e, c, j]: d = c*128 + e, j = global column
    kview = kaug_d.rearrange("(c e) j -> e c j", e=P)

    with TileContext(nc) as tc:
        with tc.tile_pool(name="kpool", bufs=3) as kpool, \
             tc.tile_pool(name="spool", bufs=2) as spool, \
             tc.tile_pool(name="cpool", bufs=1) as cpool, \
             tc.tile_pool(name="ppool", bufs=6, space="PSUM") as ppool:
            ident = cpool.tile([P, P], kdt)
            make_identity(nc, ident)
            dots_t = cpool.tile([P, BLOCKS], f32)
            norms_t = cpool.tile([P, BLOCKS], f32)

            def body():
                for p in range(panels):
                    kt = kpool.tile([P, C, pw], kdt, tag="kt", name="kt")
                    nc.sync.dma_start(
                        out=kt, in_=kview[:, :, p * pw:(p + 1) * pw]
                    )
                    for j in range(panel_blocks):
                        b = p * panel_blocks + j
                        ps = ppool.tile([P, P], f32, tag="ps", name="ps")
                        for c in range(C):
                            blk = kt[:, c, j * P:(j + 1) * P]
                            nc.tensor.matmul(
                                ps, blk, blk,
                                start=(c == 0), stop=(c == C - 1),
                            )
                        # diagonal -> norms2 (DVE); column 0 -> dots (ACT)
                        scr = spool.tile([P, P], kdt, tag="scr", name="scr")
                        nc.vector.scalar_tensor_tensor(
                            out=scr,
                            in0=ps,
                            scalar=1.0,
                            in1=ident,
                            op0=mybir.AluOpType.bypass,
                            op1=mybir.AluOpType.mult,
                            accum_out=norms_t[:, b:b + 1],
                        )
                        nc.scalar.activation(
                            dots_t[:, b:b + 1],
                            ps[:, 0:1],
                            mybir.ActivationFunctionType.Copy,
                        )

            if repeats == 1:
                body()
            else:
                with tc.For_i(0, repeats, 1):
                    body()

            nc.sync.dma_start(out=dots_d, in_=dots_t)
            nc.sync.dma_start(out=norms2_d, in_=norms_t)
    nc.compile()
    return nc


def _get_nc(repeats: int = 1, **kw):
    key = ("nc", repeats, tuple(sorted(kw.items())))
    if key not in _CACHE:
        _CACHE[key] = _build_bass(repeats, **kw)
    return _CACHE[key]


def _make_in_maps(keys: np.ndarray, query: np.ndarray,
                  fp16: bool = USE_FP16):
    """Per-core augmented transposed buffers [D, BLOCKS*128] where each
    128-col block is [q | 127 key rows]."""
    dt = np.float16 if fp16 else np.float32
    q16 = query.astype(dt)
    in_maps = []
    for i in range(N_CORES):
        lo = i * ROWS_PER_CORE
        n_real = min(ROWS_PER_CORE, max(0, N - lo))
        shard = np.zeros((BLOCKS * RPB, D), dtype=dt)
        shard[:n_real] = keys[lo:lo + n_real].astype(dt)
        aug = np.empty((BLOCKS, P, D), dtype=dt)
        aug[:, 0, :] = q16
        aug[:, 1:, :] = shard.reshape(BLOCKS, RPB, D)
        kaug = np.ascontiguousarray(aug.reshape(COLS, D).T)
        in_maps.append({"kaug": kaug})
    return in_maps


def _run_device(keys: np.ndarray, query: np.ndarray, trace: bool = False):
    """Run the SPMD kernel; returns (dots[100352], norms2[100352], results)."""
    nc = _get_nc()
    in_maps = _make_in_maps(keys, query)
    res = run_bass_kernel_spmd(
        nc, in_maps, core_ids=list(range(N_CORES)), trace=trace
    )
    dots = np.empty(N_CORES * ROWS_PER_CORE, np.float32)
    norms2 = np.empty(N_CORES * ROWS_PER_CORE, np.float32)
    for i, out in enumerate(res.results):
        # out["dots"][m, b] is <q, column m of block b>; column m>=1 of
        # block b holds key row b*127 + (m-1) of shard i
        base = i * ROWS_PER_CORE
        d = out["dots"][1:, :].T.reshape(-1)[:ROWS_PER_CORE]
        n2 = out["norms2"][1:, :].T.reshape(-1)[:ROWS_PER_CORE]
        dots[base:base + ROWS_PER_CORE] = d
        norms2[base:base + ROWS_PER_CORE] = n2
    return dots, norms2, res


def kernel(**inputs) -> np.ndarray:
    query = np.asarray(inputs["query_key"], dtype=np.float32)
    keys = np.asarray(inputs["keys"], dtype=np.float32)
    actions = np.asarray(inputs["actions"])
    top_k = int(inputs["top_k"])
    if top_k <= 0:
        return actions[:0]
    top_k = min(top_k, keys.shape[0])

    dots, norms2, _ = _run_device(keys, query)
    dots = dots[:N]
    norms2 = norms2[:N]

    q16 = query.astype(np.float16).astype(np.float32)
    q_norm = np.float32(np.linalg.norm(q16))
    denom = np.maximum(np.sqrt(norms2) * q_norm, np.float32(EPS))
    sims = dots / denom

    # Candidate set from device sims (margin >= ~10 sigma of the device
    # quantization error at T=64 already; 256 is bulletproof), then exact
    # host rescore of just those rows so the final ranking matches the
    # reference bit-for-bit regardless of device precision.
    T = min(256, N)
    cand = np.argpartition(-sims, T - 1)[:T]
    kc = keys[cand].astype(np.float64)
    qd = query.astype(np.float64)
    sc = (kc @ qd) / np.maximum(
        np.sqrt((kc * kc).sum(-1)) * np.linalg.norm(qd), EPS
    )
    # top_k, ties resolved to the lower index (jax.lax.top_k semantics)
    order = np.lexsort((cand, -sc))[:top_k]
    idx = cand[order]
    return actions[idx]


# revision 30
# speedup vs baseline: 2.2289x; 2.2289x over previous
"""Sharded cosine-similarity kNN retrieval kernel for Trainium2 (Bass/Tile).

Problem: one query [D] against keys [N, D]; return actions[top_k indices of
cosine similarity].  N=100000, D=2048, A=7, top_k<=8.

Strategy (v3, fp8 hybrid PE-Gram + DVE/ACT row path):
  - Shard keys row-wise across 8 NeuronCores; all key data is cast to fp8
    (e4m3) on the host, halving HBM traffic vs fp16.  Exactness of the final
    top-k is restored by an exact host-side rescore of the top-256
    device-sim candidates (margin ~20 sigma of the fp8 quantization error).
  - Each core's shard is split between two concurrent pipelines:
    * PE path (B_PE blocks of 127 keys): HOST-TRANSPOSED panel-major buffer
      kaug [panels, e, c, j] where each 128-column block is [q | 127 keys]
      (q embedded at column 0).  Per block, one accumulation group of 16
      matmuls  psum += B_c.T @ B_c  (lhsT = rhs = 128-d chunk of the block)
      yields BOTH dots (psum column 0) and norms^2 (psum diagonal) in a
      single TensorE stream.  DVE extracts the diagonal (identity-mask
      multiply-accumulate), ACT copies the dots column.
    * Row path (T_ROW tiles of 128 keys): row-major buffer; DVE computes
      dots via fused multiply-accumulate against a broadcast q tile, ACT
      computes norms^2 via Square-activation with accumulate.
    The split ratio balances TensorE vs DVE vs ACT at ~80 us each, just
    above the ~77 us DMA roofline for the fp8 stream.
  - Host: sims = dots / max(|k| * |q|, eps); top-256 candidates by device
    sims; exact fp64 rescore of candidates; return actions[top_k].
"""

import sys

for _p in ("/opt/trn_rl_repo", "/opt/trn_rl_repo/concourse"):
    if _p not in sys.path:
        sys.path.insert(0, _p)

import numpy as np

import concourse.bacc as bacc
from concourse import mybir
from concourse.bass_utils import run_bass_kernel_spmd
from concourse.masks import make_identity
from concourse.tile import TileContext

N, D, A = 100000, 2048, 7
EPS = 1e-8
N_CORES = 8
P = 128
C = D // P                       # 16 d-chunks
ROWS_PER_CORE = 12544            # 8*12544 = 100352 >= N

# row path (DVE dots + ACT norms; a few dot tiles on GPSIMD+ACT)
T_ROW = 29                       # 128-row tiles
ROW_ROWS = T_ROW * P             # 3712
ROW_CHUNK = 3                    # tiles per DMA
POOL_DOTS = 0                    # row-dot tiles via GPSIMD mult + ACT reduce

# PE path (Gram blocks of [q | 127 keys])
RPB = 127                        # key rows per 128-col block
B_PE = 70                       # blocks; 70*127 = 8890 >= 12544-3712
PE_ROWS = ROWS_PER_CORE - ROW_ROWS   # 8832 real rows
PANEL_BLOCKS = 7                 # blocks per DMA panel
PANELS = B_PE // PANEL_BLOCKS    # 10

KDT = "fp8"                      # "fp16" | "fp8" (e4m3) key/query dtype

_CACHE = {}


def _np_kdt(kdt_name: str):
    if kdt_name == "fp8":
        import ml_dtypes
        return ml_dtypes.float8_e4m3
    return np.float16


def _mybir_kdt(kdt_name: str):
    return (mybir.dt.float8e4 if kdt_name == "fp8"
            else mybir.dt.float16)


def _row_chunks(t_row=T_ROW, chunk=ROW_CHUNK):
    out = []
    t = 0
    while t < t_row:
        out.append((t, min(chunk, t_row - t)))
        t += chunk
    return out


def _build_bass(repeats: int = 1, kdt_name: str = KDT,
                t_row: int = T_ROW, b_pe: int = B_PE,
                panel_blocks: int = PANEL_BLOCKS, row_chunk: int = ROW_CHUNK,
                pool_dots: int = POOL_DOTS, mode: str = "full"):
    """Build the per-core Bass program.

    repeats>1 wraps the streaming loop in a hardware For loop that re-reads
    the same DRAM shards; used only for wall-clock HW timing (slope over
    repeats cancels host/axon dispatch overhead).

    mode: 'full' | 'dma' (no compute) | 'pe' (PE pipeline only) |
          'row' (row pipeline only)."""
    assert b_pe % panel_blocks == 0
    panels = b_pe // panel_blocks
    pw = panel_blocks * P
    chunks = _row_chunks(t_row, row_chunk)

    nc = bacc.Bacc(
        "TRN2",
        target_bir_lowering=False,
        debug=False,
        enable_asserts=False,
        num_devices=N_CORES,
    )
    f32 = mybir.dt.float32
    f16 = mybir.dt.float16
    kdt = _mybir_kdt(kdt_name)
    kaug_d = nc.dram_tensor(
        "kaug", [panels, P, C, pw], kdt, kind="ExternalInput"
    ).ap()
    krow_d = nc.dram_tensor(
        "krow", [t_row * P, D], kdt, kind="ExternalInput"
    ).ap()
    qb_d = nc.dram_tensor("qb", [P, D], kdt, kind="ExternalInput").ap()
    dots_g_d = nc.dram_tensor(
        "dots_g", [P, b_pe], f32, kind="ExternalOutput"
    ).ap()
    norms_g_d = nc.dram_tensor(
        "norms_g", [P, b_pe], f32, kind="ExternalOutput"
    ).ap()
    dots_r_d = nc.dram_tensor(
        "dots_r", [P, t_row], f32, kind="ExternalOutput"
    ).ap()
    norms_r_d = nc.dram_tensor(
        "norms_r", [P, t_row], f32, kind="ExternalOutput"
    ).ap()

    # krow viewed as [p, t, d]: row t*128+p -> partition p, tile t
    krow_r = krow_d.rearrange("(t p) d -> p t d", p=P)

    do_pe = mode in ("full", "pe", "dma")
    do_row = mode in ("full", "row", "dma")

    with TileContext(nc) as tc:
        with tc.tile_pool(name="kpool", bufs=3) as kpool, \
             tc.tile_pool(name="rpool", bufs=3) as rpool, \
             tc.tile_pool(name="spool", bufs=2) as spool, \
             tc.tile_pool(name="cpool", bufs=1) as cpool, \
             tc.tile_pool(name="ppool", bufs=8, space="PSUM") as ppool:
            ident = cpool.tile([P, P], f16)
            make_identity(nc, ident)
            qb_t = cpool.tile([P, D], kdt)
            nc.sync.dma_start(out=qb_t, in_=qb_d)
            dots_g_t = cpool.tile([P, b_pe], f32)
            norms_g_t = cpool.tile([P, b_pe], f32)
            dots_r_t = cpool.tile([P, t_row], f32)
            norms_r_t = cpool.tile([P, t_row], f32)
            if mode != "full":
                for t_ in (dots_g_t, norms_g_t, dots_r_t, norms_r_t):
                    nc.vector.memset(t_, 0.0)

            def pe_block(kt, j, b):
                ps = ppool.tile([P, P], f32, tag="ps", name="ps")
                for c in range(C):
                    blk = kt[:, c, j * P:(j + 1) * P]
                    nc.tensor.matmul(
                        ps, blk, blk,
                        start=(c == 0), stop=(c == C - 1),
                    )
                # diagonal -> norms2 (DVE); column 0 -> dots (ACT).
                # High priority: these free PSUM banks, so they must win
                # queue slots over the 2.4us row-path ops or the PE stalls.
                with tc.high_priority():
                    scr = spool.tile([P, P], f16, tag="scr", name="scr")
                    nc.vector.scalar_tensor_tensor(
                        out=scr,
                        in0=ps,
                        scalar=1.0,
                        in1=ident,
                        op0=mybir.AluOpType.bypass,
                        op1=mybir.AluOpType.mult,
                        accum_out=norms_g_t[:, b:b + 1],
                    )
                    nc.scalar.activation(
                        dots_g_t[:, b:b + 1],
                        ps[:, 0:1],
                        mybir.ActivationFunctionType.Copy,
                    )

            pool_set = set(range(t_row - 1, -1, -max(1, t_row // max(
                pool_dots, 1)))[:pool_dots]) if pool_dots else set()

            def row_tile(rt, u, t):
                if t in pool_set:
                    # offload: GPSIMD multiply, ACT copy-accum reduce
                    pp_ = spool.tile([P, D], f16, tag="pp", name="pp")
                    nc.gpsimd.tensor_tensor(
                        pp_, rt[:, u], qb_t, mybir.AluOpType.mult
                    )
                    cp_ = spool.tile([P, D], f16, tag="cp", name="cp")
                    nc.scalar.activation(
                        cp_,
                        pp_,
                        mybir.ActivationFunctionType.Copy,
                        accum_out=dots_r_t[:, t:t + 1],
                    )
                else:
                    prod = spool.tile([P, D], f16, tag="prod", name="prod")
                    nc.vector.scalar_tensor_tensor(
                        out=prod,
                        in0=rt[:, u],
                        scalar=1.0,
                        in1=qb_t,
                        op0=mybir.AluOpType.bypass,
                        op1=mybir.AluOpType.mult,
                        accum_out=dots_r_t[:, t:t + 1],
                    )
                sq = spool.tile([P, D], f16, tag="sq", name="sq")
                nc.scalar.activation(
                    sq,
                    rt[:, u],
                    mybir.ActivationFunctionType.Square,
                    accum_out=norms_r_t[:, t:t + 1],
                )

            def body():
                ci = 0
                iters = max(panels, len(chunks))
                for p in range(iters):
                    kt = None
                    if do_pe and p < panels:
                        kt = kpool.tile([P, C, pw], kdt, tag="kt", name="kt")
                        nc.sync.dma_start(out=kt, in_=kaug_d[p])
                    rt, t0, cnt = None, 0, 0
                    # spread row chunks evenly across the panel iterations
                    want = ((p + 1) * len(chunks) + iters - 1) // iters
                    if do_row and ci < min(want, len(chunks)):
                        t0, cnt = chunks[ci]
                        ci += 1
                        rt = rpool.tile([P, row_chunk, D], kdt, tag="rt",
                                        name="rt")
                        nc.sync.dma_start(
                            out=rt[:, :cnt], in_=krow_r[:, t0:t0 + cnt]
                        )
                    if mode == "dma":
                        continue
                    # interleave row tiles between PE blocks so the shared
                    # DVE/ACT queues alternate between the two pipelines
                    u = 0
                    for j in range(panel_blocks if kt is not None else 0):
                        pe_block(kt, j, p * panel_blocks + j)
                        if rt is not None and u < cnt and j % 2 == 1:
                            row_tile(rt, u, t0 + u)
                            u += 1
                    while rt is not None and u < cnt:
                        row_tile(rt, u, t0 + u)
                        u += 1

            if repeats == 1:
                body()
            else:
                with tc.For_i(0, repeats, 1):
                    body()

            nc.sync.dma_start(out=dots_g_d, in_=dots_g_t)
            nc.sync.dma_start(out=norms_g_d, in_=norms_g_t)
            nc.sync.dma_start(out=dots_r_d, in_=dots_r_t)
            nc.sync.dma_start(out=norms_r_d, in_=norms_r_t)
    nc.compile()
    return nc


def _get_nc(repeats: int = 1, **kw):
    key = ("nc", repeats, tuple(sorted(kw.items())))
    if key not in _CACHE:
        _CACHE[key] = _build_bass(repeats, **kw)
    return _CACHE[key]


def _make_in_maps(keys: np.ndarray, query: np.ndarray,
                  kdt_name: str = KDT,
                  t_row: int = T_ROW, b_pe: int = B_PE,
                  panel_blocks: int = PANEL_BLOCKS):
    """Per-core inputs: row-major krow (first t_row*128 rows), panel-major
    q-augmented transposed kaug (remaining rows), broadcast q tile."""
    dt = _np_kdt(kdt_name)
    panels = b_pe // panel_blocks
    pw = panel_blocks * P
    row_rows = t_row * P
    q8 = query.astype(dt)
    qb = np.ascontiguousarray(np.broadcast_to(q8, (P, D)))
    in_maps = []
    for i in range(N_CORES):
        lo = i * ROWS_PER_CORE
        n_real = min(ROWS_PER_CORE, max(0, N - lo))
        shard = np.zeros((max(ROWS_PER_CORE, row_rows + b_pe * RPB), D),
                         dtype=dt)
        shard[:n_real] = keys[lo:lo + n_real].astype(dt)
        krow = np.ascontiguousarray(shard[:row_rows])
        pe_rows = shard[row_rows:row_rows + b_pe * RPB]
        aug = np.empty((b_pe, P, D), dtype=dt)
        aug[:, 0, :] = q8
        aug[:, 1:, :] = pe_rows.reshape(b_pe, RPB, D)
        # aug[(p jb), m, (c e)] -> kaug[p, e, c, (jb m)]
        a5 = aug.reshape(panels, panel_blocks, P, C, P)
        kaug = np.ascontiguousarray(
            a5.transpose(0, 4, 3, 1, 2)
        ).reshape(panels, P, C, pw)
        in_maps.append({"kaug": kaug, "krow": krow, "qb": qb})
    return in_maps


def _run_device(keys: np.ndarray, query: np.ndarray, trace: bool = False):
    """Run the SPMD kernel; returns (dots[100352], norms2[100352], results)."""
    nc = _get_nc()
    in_maps = _make_in_maps(keys, query)
    res = run_bass_kernel_spmd(
        nc, in_maps, core_ids=list(range(N_CORES)), trace=trace
    )
    dots = np.empty(N_CORES * ROWS_PER_CORE, np.float32)
    norms2 = np.empty(N_CORES * ROWS_PER_CORE, np.float32)
    pe_rows = ROWS_PER_CORE - ROW_ROWS
    for i, out in enumerate(res.results):
        base = i * ROWS_PER_CORE
        # row path: dots_r[p, t] is row t*128+p of the shard
        dots[base:base + ROW_ROWS] = out["dots_r"].T.reshape(-1)
        norms2[base:base + ROW_ROWS] = out["norms_r"].T.reshape(-1)
        # PE path: dots_g[m, b] (m>=1) is row ROW_ROWS + b*127 + (m-1)
        d = out["dots_g"][1:, :].T.reshape(-1)[:pe_rows]
        n2 = out["norms_g"][1:, :].T.reshape(-1)[:pe_rows]
        dots[base + ROW_ROWS:base + ROWS_PER_CORE] = d
        norms2[base + ROW_ROWS:base + ROWS_PER_CORE] = n2
    return dots, norms2, res


def kernel(**inputs) -> np.ndarray:
    query = np.asarray(inputs["query_key"], dtype=np.float32)
    keys = np.asarray(inputs["keys"], dtype=np.float32)
    actions = np.asarray(inputs["actions"])
    top_k = int(inputs["top_k"])
    if top_k <= 0:
        return actions[:0]
    top_k = min(top_k, keys.shape[0])

    dots, norms2, _ = _run_device(keys, query)
    dots = dots[:N]
    norms2 = norms2[:N]

    qd_dev = query.astype(_np_kdt(KDT)).astype(np.float32)
    q_norm = np.float32(np.linalg.norm(qd_dev))
    denom = np.maximum(np.sqrt(norms2) * q_norm, np.float32(EPS))
    sims = dots / denom

    # Candidate set from device sims (margin >= ~10 sigma of the device
    # quantization error at T=64 already; 256 is bulletproof), then exact
    # host rescore of just those rows so the final ranking matches the
    # reference regardless of device precision.
    T = min(256, N)
    cand = np.argpartition(-sims, T - 1)[:T]
    kc = keys[cand].astype(np.float64)
    qd = query.astype(np.float64)
    sc = (kc @ qd) / np.maximum(
        np.sqrt((kc * kc).sum(-1)) * np.linalg.norm(qd), EPS
    )
    # top_k, ties resolved to the lower index (jax.lax.top_k semantics)
    order = np.lexsort((cand, -sc))[:top_k]
    idx = cand[order]
    return actions[idx]
